# revision 65
# baseline (speedup 1.0000x reference)
"""MoE FFN (shared SwiGLU + 8 dense-routed SwiGLU experts) on 8 TRN2 NeuronCores.

Strategy: data-parallel over batch (B=16 -> 2 batches/core). The 10 uniform
512->1024->512 SwiGLU units (2 shared halves + 8 experts) run with per-unit
precision: shared units in bf16, expert units in fp8e4m3 DoubleRow matmuls
(2x PE throughput). Expert quantization errors are independent across the 8
experts and average down under the routing weights, keeping total rel err
~1.8e-2 (<2e-2 gate) while cutting PE time to ~0.6x of the bf16 roofline.

All weights stay resident in SBUF (~170KB/partition incl. x), host-repacked
to partition-contiguous layout. Loop is token-tile-outer / unit-inner so all
10 units' down-projections accumulate directly in PSUM at a common scale
(shared h pre-scaled by SW*C=32768, exact in floating point); one op per
(d-tile, token-tile) drains PSUM with bias + descale fused, alternating
between the scalar and vector engines. With zero up-biases (this problem's
inputs) the whole up-branch scale + h product is a single fused
scalar_tensor_tensor on the DVE, so each h-tile costs one scalar-engine op
(silu) and one DVE op.

Startup (measured DMA behavior): no payload moves before ~8.8us, and all
descriptors queued on a ring transfer CONCURRENTLY, so landing time is
governed by total queued bytes (~140-210GB/s early). Hence: (1) a tiny wz
tensor lands first and feeds 32 dummy matmuls that open the HAM clock-gate
window (PE reaches 2.4GHz during warmup instead of ~24.6us in, when early
DMA-stall gaps kept resetting the 3.4us sustained-busy requirement); (2)
only the first two units' gate/up weights are queued upfront in consumption
order; (3) every later unit's weights (and each bulk x/xb chunk) are
released just-in-time from the t=0 compute loop, gated on the previous
unit's first silu/stt via an 8-byte dummy DMA, keeping the in-flight set to
~1.5MB/window so payloads land in order one window ahead of use; (4) unit 0
runs gate-pass-then-up-pass so its gate weights alone unblock the PE. The
final token tile closes its PSUM accumulation d-tile-major and splits drains
across both engines. Measured: 519.6us (vs 525.2us before this round, 853us
bf16 baseline); PE busy ~499.8us vs the 491.5us mixed-precision roofline
(the residual is a fixed ~0.38us stall every ~10.8us, believed HBM refresh),
rel err 1.841e-2. Caution when re-benching: the part thermally throttles
2.4->2.0GHz across back-to-back runs (steady matmul slice 216ns -> 259ns);
normalize before comparing.
"""
import sys

if "/opt/trn_rl_repo" not in sys.path:
    sys.path.insert(0, "/opt/trn_rl_repo")

import numpy as np
import ml_dtypes

import concourse.bass as bass  # noqa: F401  (registers engine classes)
import concourse.tile as tile
from concourse import bacc, mybir
from concourse import bass_utils

F32 = mybir.dt.float32
BF16 = mybir.dt.bfloat16
FP8 = mybir.dt.float8e4
Silu = mybir.ActivationFunctionType.Silu
ACT = Silu  # debug harnesses may swap to Sigmoid (CoreSim lacks Silu)
Alu = mybir.AluOpType
DR = mybir.MatmulPerfMode.DoubleRow

B, K, D = 16, 1024, 512
HS, HR, E = 2048, 1024, 8
NCORES = 8
BL = B // NCORES          # batches per core = 2
T = BL * K                # tokens per core = 2048
TT = 512                  # token tile (matmul moving dim)
NTT = T // TT             # 4 token tiles per core
NU = 2 + E                # units: 2 shared halves + 8 experts
HU = 1024                 # hidden width of every unit
NH = HU // 128            # 8 h-tiles per unit
ND = D // 128             # 4 d-tiles
NK = D // 128             # 4 contraction tiles for gate/up
WSZ = NK * HU             # per-matrix elements per partition (4096)

SX = 32.0                 # fp8 x scale
SWQ = 1024.0              # fp8 weight scale
C = 32.0                  # h-domain scale for expert fp8 h
SXW = SX * SWQ            # expert gate/up psum scale
SDC = SWQ * C             # common down psum scale (all units)

# per-unit precision: shared halves bf16, experts fp8
UNIT_FP8 = [False, False] + [True] * E
# experts first: the first unit's weights (1MB fp8 vs 3MB bf16) gate PE start
UORDER = list(range(2, NU)) + [0, 1]

# When every up-projection bias is zero (true for this problem's inputs),
# the up-branch scale and the h product fuse into one DVE op:
#   h = (ups * rwC) * silu(g).
# The general path (ts on vector + tt on gpsimd) stays available for
# nonzero biases; _run picks at call time.


def _build(fused):
    nc = bacc.Bacc("TRN2", target_bir_lowering=False, debug=False,
                   num_devices=NCORES)
    # weights packed host-side to SBUF layout: partition-contiguous, one
    # (unit, matrix) block of WSZ elements per partition per block.
    wsb = nc.dram_tensor("wsb", (128, 2 * 3 * WSZ), BF16, kind="ExternalInput")
    we8 = nc.dram_tensor("we8", (128, E * 3 * WSZ), FP8, kind="ExternalInput")
    xTb = nc.dram_tensor("xTb", (128, NK, T), BF16, kind="ExternalInput")
    xT8 = nc.dram_tensor("xT8", (128, NK, T), FP8, kind="ExternalInput")
    gb = nc.dram_tensor("gb", (128, NU, NH), F32, kind="ExternalInput")
    ub = nc.dram_tensor("ub", (128, NU, NH), F32, kind="ExternalInput")
    rw = nc.dram_tensor("rw", (128, NU, NTT), F32, kind="ExternalInput")
    cv = nc.dram_tensor("cv", (128, ND, NTT), F32, kind="ExternalInput")
    wz = nc.dram_tensor("wz", (128, 128), BF16, kind="ExternalInput")
    outT = nc.dram_tensor("outT", (D, T), F32, kind="ExternalOutput")

    with tile.TileContext(nc) as tc:
        with (
            tc.tile_pool(name="persist", bufs=1) as persist,
            # non-fused builds need the su tiles; single-buffered h keeps
            # them inside SBUF (the fallback just serializes on h reuse)
            tc.tile_pool(name="hpool", bufs=2 if fused else 1) as hpool,
            tc.tile_pool(name="spool", bufs=2) as spool,
            tc.tile_pool(name="dpool", bufs=2) as dpool,
            tc.tile_pool(name="gups", bufs=2, space="PSUM") as gups,
            tc.tile_pool(name="ops", bufs=1, space="PSUM") as opsp,
        ):
            xb = persist.tile([128, NK, T], BF16)
            x8t = persist.tile([128, NK, T], FP8)
            gbt = persist.tile([128, NU, NH], F32)
            # ubt is only read on the non-fused path; skip it (and its DMA)
            # in fused builds for SBUF headroom
            ubt = None if fused else persist.tile([128, NU, NH], F32)
            rwt = persist.tile([128, NU, NTT], F32)
            cvt = persist.tile([128, ND, NTT], F32)
            wzt = persist.tile([128, 128], BF16, name="wzt")

            # PE pstate warmup: the HAM clock gate holds the PE at 1.2GHz
            # until it sees ~3.4us of *sustained* busy; any DMA-wait gap in
            # the early stream resets that window. Dummy matmuls over the
            # first-landed wz tile keep the PE busy from ~9.5us until the
            # first unit's weights arrive (~13us), so the un-throttle fires
            # during warmup and the real stream runs warm almost from its
            # first instruction. Fine granularity (free=128) keeps the
            # overrun past weight-landing small. Accumulator never read.
            wout = persist.tile([128, 256], BF16, name="wout")
            wps = gups.tile([128, TT], F32, tag="g", name="warmps")
            # unit-0 runs gate-pass-then-up-pass (so the gate weights alone
            # unblock the PE while the up weights are still in flight); the
            # 8 silu results need 8 live buffers instead of spool's 2. fp8
            # (for SBUF headroom) costs ~0.1% extra on 1 of 40 unit-windows.
            sg0 = persist.tile([128, NH, TT], FP8, name="sg0")

            # DMA facts (measured): payloads only start flowing ~8.8us, a
            # ring's queued descriptors transfer CONCURRENTLY (so completion
            # smears across everything queued, ~140-210GB/s aggregate early).
            # Strategy: queue upfront only what the first two units need, in
            # consumption order, then release each later unit's weights (and
            # one bulk x/xb chunk) just-in-time from inside the compute loop,
            # gated on the previous unit's first silu/stt via a tiny dummy
            # transfer. This keeps the in-flight set small so payloads land
            # in order, one window ahead of use.
            wtiles, wsrcs = {}, {}
            for idx, u in enumerate(UORDER):
                fp8u = UNIT_FP8[u]
                dt_ = FP8 if fp8u else BF16
                src = we8 if fp8u else wsb
                base = (u - 2 if fp8u else u) * 3 * WSZ
                wgt = persist.tile([128, NK, HU], dt_, name=f"wg{u}")
                wut = persist.tile([128, NK, HU], dt_, name=f"wu{u}")
                wdt = persist.tile([128, NH, D], dt_, name=f"wd{u}")
                wtiles[u] = (wgt, wut, wdt)
                wsrcs[u] = (src, base)
                if idx == 0:
                    nc.gpsimd.dma_start(x8t[:, :, 0:TT], xT8.ap()[:, :, 0:TT])
                    nc.gpsimd.dma_start(wgt[:], src.ap()[:, base:base + WSZ])
                    nc.gpsimd.dma_start(wut[:],
                                        src.ap()[:, base + WSZ:base + 2 * WSZ])
                    nc.sync.dma_start(wzt[:], wz.ap()[:])
                    nc.sync.dma_start(gbt[:], gb.ap()[:])
                    nc.sync.dma_start(rwt[:], rw.ap()[:])
                    if not fused:
                        nc.sync.dma_start(ubt[:], ub.ap()[:])
                    nc.sync.dma_start(cvt[:], cv.ap()[:])
                    # PE warmup: wz (32KB) lands ~9.4us (first sync payload);
                    # matmuls over it keep the PE busy until the gate weights
                    # land (~13us), so the HAM un-throttle to 2.4GHz fires
                    # ~12.8us in and the real stream runs warm almost from
                    # its first instruction. Accumulator never read.
                    for i in range(32):
                        nc.tensor.matmul(wps[:, 0:128], wzt[:, 0:128],
                                         wzt[:],
                                         start=(i == 0), stop=(i == 31))
                    nc.scalar.activation(wout[:], x8t[:, 0, 0:256], ACT)
                    nc.vector.scalar_tensor_tensor(
                        wout[:], x8t[:, 0, 0:256], 1.0, x8t[:, 1, 0:256],
                        Alu.mult, Alu.mult)
                    nc.scalar.activation(wout[:], x8t[:, 1, 0:256], ACT)
                    nc.vector.scalar_tensor_tensor(
                        wout[:], x8t[:, 1, 0:256], 1.0, x8t[:, 0, 0:256],
                        Alu.mult, Alu.mult)
                elif idx == 1 and fused:
                    # second unit's gate+up ride upfront too (its window
                    # starts ~20us, before the first in-loop gate can
                    # supply). The up matrix goes via sync: it is not
                    # start-critical (needed ~21us; sync delivers by ~15
                    # even on a late ring start) and dropping it from the
                    # gpsimd in-flight set lands unit-0's up weights sooner.
                    nc.gpsimd.dma_start(wgt[:], src.ap()[:, base:base + WSZ])
                    nc.sync.dma_start(wut[:],
                                      src.ap()[:, base + WSZ:base + 2 * WSZ])
                elif not fused:
                    q = nc.gpsimd if fp8u else nc.sync
                    for wt, off in [(wgt, 0), (wut, WSZ), (wdt, 2 * WSZ)]:
                        q.dma_start(wt[:],
                                    src.ap()[:, base + off:base + off + WSZ])
            if not fused:
                nc.sync.dma_start(xb[:, :, 0:TT], xTb.ap()[:, :, 0:TT])
                nc.sync.dma_start(x8t[:, :, TT:], xT8.ap()[:, :, TT:])
                nc.sync.dma_start(xb[:, :, TT:], xTb.ap()[:, :, TT:])

            def wdma(q, u, m):
                src, base = wsrcs[u]
                q.dma_start(
                    wtiles[u][m][:],
                    src.ap()[:, base + m * WSZ:base + (m + 1) * WSZ])

            U = UORDER
            # releases[k]: weight DMAs to issue once unit k's first silu has
            # run: next unit's gate+up, current unit's down (wd is consumed
            # one window later by the software-pipelined down matmuls).
            # Gate 0's releases ride the sync ring: unit 0's pass-2 needs
            # the gpsimd engine for its alternating stts.
            releases = {0: [(U[0], 2)], 9: [(U[9], 2)]}
            for k in range(1, 9):
                releases[k] = [(U[k + 1], 0), (U[k + 1], 1), (U[k], 2)]
            # bulk x/xb chunks, one per gate: all needed >=90us in, so any
            # landing time is fine — the gating only bounds their burst size.
            bulkchunks = [
                (xb[:, 0:2, 0:TT], xTb.ap()[:, 0:2, 0:TT]),
                (xb[:, 2:4, 0:TT], xTb.ap()[:, 2:4, 0:TT]),
                (x8t[:, :, TT:2 * TT], xT8.ap()[:, :, TT:2 * TT]),
                (x8t[:, :, 2 * TT:3 * TT], xT8.ap()[:, :, 2 * TT:3 * TT]),
                (x8t[:, :, 3 * TT:], xT8.ap()[:, :, 3 * TT:]),
                (xb[:, :, TT:2 * TT], xTb.ap()[:, :, TT:2 * TT]),
                (xb[:, :, 2 * TT:3 * TT], xTb.ap()[:, :, 2 * TT:3 * TT]),
                (xb[:, :, 3 * TT:], xTb.ap()[:, :, 3 * TT:]),
            ]
            scrtG = persist.tile([128, 8], FP8, name="scrtG")
            scrtGb = persist.tile([128, 8], BF16, name="scrtGb")
            scrtS = persist.tile([128, 8], FP8, name="scrtS")

            def emit_gates(k, gsrc, gsrc_fp8):
                if k == 0:
                    nc.sync.dma_start(scrtS[:], gsrc)
                    for uu, m in releases[0]:
                        wdma(nc.sync, uu, m)
                    nc.sync.dma_start(*bulkchunks[0])
                    return
                if releases.get(k):
                    nc.gpsimd.dma_start(
                        (scrtG if gsrc_fp8 else scrtGb)[:], gsrc)
                    for uu, m in releases[k]:
                        wdma(nc.gpsimd, uu, m)
                if k < len(bulkchunks):
                    nc.sync.dma_start(scrtS[:], gsrc)
                    nc.sync.dma_start(*bulkchunks[k])

            # The PE stream is software-pipelined by one unit: unit (t,ui)'s
            # down matmuls are emitted after unit (t,ui+1)'s gate/up, so the
            # silu->stt h-chain always has a full unit window (10-20us) of
            # PE cover instead of ~6us — this removes the pipeline-fill
            # stalls (and their pstate drops) in the first ~30us and hides
            # the t-boundary drains. Requires hpool bufs=2.
            odsts = {}

            def emit_down(t, ui, fp8u, wdt, hts):
                if ui == 0:
                    odsts[t] = [opsp.tile([128, TT], F32, tag=f"o{di}",
                                          name=f"o{di}_t{t}")
                                for di in range(ND)]
                odst = odsts[t]
                last = ui == NU - 1
                # on the very last unit of the kernel, close each d-tile's
                # accumulation group early (di-major) so the final drains
                # overlap the remaining down matmuls
                dimaj = last and t == NTT - 1
                if fp8u:
                    kds = ([(kp, di) for di in range(ND)
                            for kp in range(NH // 2)] if dimaj else
                           [(kp, di) for kp in range(NH // 2)
                            for di in range(ND)])
                    for kp, di in kds:
                        nc.tensor.matmul(
                            odst[di][:],
                            wdt[:, 2 * kp:2 * kp + 2,
                                di * 128:(di + 1) * 128],
                            hts[:, 2 * kp:2 * kp + 2, :],
                            start=(ui == 0 and kp == 0),
                            stop=(last and kp == NH // 2 - 1),
                            perf_mode=DR, skip_group_check=True)
                else:
                    kds = ([(k, di) for di in range(ND)
                            for k in range(NH)] if dimaj else
                           [(k, di) for k in range(NH)
                            for di in range(ND)])
                    for k, di in kds:
                        nc.tensor.matmul(
                            odst[di][:],
                            wdt[:, k, di * 128:(di + 1) * 128],
                            hts[:, k, :],
                            start=(ui == 0 and k == 0),
                            stop=(last and k == NH - 1),
                            skip_group_check=True)
                if last:
                    for di in range(ND):
                        dp = slice(di * 128, (di + 1) * 128)
                        dtl = dpool.tile([128, TT], F32, tag="d")
                        if dimaj:
                            # final tile: split each drain into half-token
                            # chunks on both engines so the last out-DMA
                            # starts ~0.7us earlier
                            h0, h1 = slice(0, TT // 2), slice(TT // 2, TT)
                            nc.scalar.activation(
                                dtl[:, h0], odst[di][:, h0],
                                mybir.ActivationFunctionType.Identity,
                                bias=cvt[:, di, t:t + 1], scale=1.0 / SDC)
                            nc.vector.tensor_scalar(dtl[:, h1],
                                                    odst[di][:, h1],
                                                    1.0 / SDC,
                                                    cvt[:, di, t:t + 1],
                                                    Alu.mult, Alu.add)
                            nc.sync.dma_start(
                                outT.ap()[dp, t * TT:t * TT + TT // 2],
                                dtl[:, h0])
                            nc.sync.dma_start(
                                outT.ap()[dp, t * TT + TT // 2:(t + 1) * TT],
                                dtl[:, h1])
                            continue
                        tok = slice(t * TT, (t + 1) * TT)
                        if di % 2 == 0:
                            nc.scalar.activation(
                                dtl[:], odst[di][:],
                                mybir.ActivationFunctionType.Identity,
                                bias=cvt[:, di, t:t + 1], scale=1.0 / SDC)
                        else:
                            nc.vector.tensor_scalar(dtl[:], odst[di][:],
                                                    1.0 / SDC,
                                                    cvt[:, di, t:t + 1],
                                                    Alu.mult, Alu.add)
                        nc.sync.dma_start(
                            outT.ap()[dp, tok], dtl[:])

            pending = None
            for t in range(NTT):
                tok = slice(t * TT, (t + 1) * TT)
                for ui, u in enumerate(UORDER):
                    fp8u = UNIT_FP8[u]
                    wgt, wut, wdt = wtiles[u]
                    sdt = BF16 if fp8u else F32
                    hts = hpool.tile([128, NH, TT], FP8 if fp8u else BF16,
                                     tag="h8" if fp8u else "hb",
                                     name=f"h_u{u}t{t}")
                    if fused and t == 0 and ui < 1:
                        # First unit: all gate matmuls (+ silu into the
                        # dedicated sg0 buffer), THEN all up matmuls (+ stt).
                        # The gate weights alone unblock the PE ~2.5us before
                        # the up weights land. (The single-engine silu/stt
                        # chains pace each pass ~0.7us/h-tile vs 0.43 of
                        # matmul — a ~2us stall accepted for this one window;
                        # gpsimd can't take alternate stts, it cannot read
                        # PSUM.)
                        for hi in range(NH):
                            hc = slice(hi * 128, (hi + 1) * 128)
                            gps = gups.tile([128, TT], F32, tag="g")
                            for kp in range(NK // 2):
                                nc.tensor.matmul(
                                    gps[:], wgt[:, 2 * kp:2 * kp + 2, hc],
                                    x8t[:, 2 * kp:2 * kp + 2, tok],
                                    start=(kp == 0), stop=(kp == NK // 2 - 1),
                                    perf_mode=DR)
                            nc.scalar.activation(sg0[:, hi, :], gps[:], ACT,
                                                 bias=gbt[:, u, hi:hi + 1],
                                                 scale=1.0 / SXW)
                        emit_gates(ui, sg0[:, 0, 0:8], True)
                        for hi in range(NH):
                            hc = slice(hi * 128, (hi + 1) * 128)
                            ups = gups.tile([128, TT], F32, tag="u")
                            for kp in range(NK // 2):
                                nc.tensor.matmul(
                                    ups[:], wut[:, 2 * kp:2 * kp + 2, hc],
                                    x8t[:, 2 * kp:2 * kp + 2, tok],
                                    start=(kp == 0), stop=(kp == NK // 2 - 1),
                                    perf_mode=DR)
                            nc.vector.scalar_tensor_tensor(
                                hts[:, hi, :], ups[:], rwt[:, u, t:t + 1],
                                sg0[:, hi, :], Alu.mult, Alu.mult)
                        if pending is not None:
                            emit_down(*pending)
                        pending = (t, ui, fp8u, wdt, hts)
                        continue
                    for hi in range(NH):
                        hc = slice(hi * 128, (hi + 1) * 128)
                        gps = gups.tile([128, TT], F32, tag="g")
                        if fp8u:
                            for kp in range(NK // 2):
                                nc.tensor.matmul(
                                    gps[:], wgt[:, 2 * kp:2 * kp + 2, hc],
                                    x8t[:, 2 * kp:2 * kp + 2, tok],
                                    start=(kp == 0), stop=(kp == NK // 2 - 1),
                                    perf_mode=DR)
                        else:
                            for k in range(NK):
                                nc.tensor.matmul(
                                    gps[:], wgt[:, k, hc], xb[:, k, tok],
                                    start=(k == 0), stop=(k == NK - 1))
                        ups = gups.tile([128, TT], F32, tag="u")
                        if fp8u:
                            for kp in range(NK // 2):
                                nc.tensor.matmul(
                                    ups[:], wut[:, 2 * kp:2 * kp + 2, hc],
                                    x8t[:, 2 * kp:2 * kp + 2, tok],
                                    start=(kp == 0), stop=(kp == NK // 2 - 1),
                                    perf_mode=DR)
                        else:
                            for k in range(NK):
                                nc.tensor.matmul(
                                    ups[:], wut[:, k, hc], xb[:, k, tok],
                                    start=(k == 0), stop=(k == NK - 1))
                        sg = spool.tile([128, TT], sdt, tag="sg8" if fp8u else "sgb")
                        nc.scalar.activation(sg[:], gps[:], ACT,
                                             bias=gbt[:, u, hi:hi + 1],
                                             scale=(1.0 / SXW) if fp8u else 1.0)
                        if fused:
                            nc.vector.scalar_tensor_tensor(
                                hts[:, hi, :], ups[:], rwt[:, u, t:t + 1],
                                sg[:], Alu.mult, Alu.mult)
                        else:
                            su = spool.tile([128, TT], sdt,
                                            tag="su8" if fp8u else "sub")
                            nc.vector.tensor_scalar(su[:], ups[:],
                                                    ubt[:, u, hi:hi + 1],
                                                    rwt[:, u, t:t + 1],
                                                    Alu.add, Alu.mult)
                            nc.gpsimd.tensor_tensor(hts[:, hi, :], sg[:],
                                                    su[:], Alu.mult)

                    if fused and t == 0:
                        emit_gates(ui, hts[:, 0, 0:8], fp8u)
                    if pending is not None:
                        emit_down(*pending)
                    pending = (t, ui, fp8u, wdt, hts)
            emit_down(*pending)
    nc.compile()
    return nc


_NC = {}


def _get_nc(fused):
    if fused not in _NC:
        _NC[fused] = _build(fused)
    return _NC[fused]


def _bf16(a):
    return np.ascontiguousarray(np.asarray(a, np.float32)).astype(ml_dtypes.bfloat16)


def _fp8(a, scale):
    return np.ascontiguousarray(
        np.asarray(a, np.float32) * scale).astype(ml_dtypes.float8_e4m3)


def _colmaj(v):
    return np.asarray(v, np.float32).reshape(-1, 128).T


def _sbufpack(w):
    """[D_in, D_out] -> [128, (D_in/128)*D_out] partition-contiguous."""
    din, dout = w.shape
    return w.reshape(din // 128, 128, dout).transpose(1, 0, 2).reshape(128, -1)


def _pack_shared(Ws_gate, bs_gate, Ws_up, bs_up, Ws_down, bs_down,
                 Wr_gate, br_gate, Wr_up, br_up, Wr_down, br_down):
    ws = np.empty((128, 2 * 3 * WSZ), np.float32)
    we = np.empty((128, E * 3 * WSZ), np.float32)
    gbt = np.empty((128, NU, NH), np.float32)
    ubt = np.empty((128, NU, NH), np.float32)
    for u in range(2):
        h0 = slice(u * HU, (u + 1) * HU)
        base = u * 3 * WSZ
        ws[:, base:base + WSZ] = _sbufpack(np.asarray(Ws_gate, np.float32)[:, h0])
        ws[:, base + WSZ:base + 2 * WSZ] = _sbufpack(
            np.asarray(Ws_up, np.float32)[:, h0])
        ws[:, base + 2 * WSZ:base + 3 * WSZ] = _sbufpack(
            np.asarray(Ws_down, np.float32)[h0, :])
        gbt[:, u, :] = _colmaj(bs_gate[h0])
        ubt[:, u, :] = _colmaj(bs_up[h0])
    for e in range(E):
        base = e * 3 * WSZ
        we[:, base:base + WSZ] = _sbufpack(np.asarray(Wr_gate, np.float32)[e])
        we[:, base + WSZ:base + 2 * WSZ] = _sbufpack(
            np.asarray(Wr_up, np.float32)[e])
        we[:, base + 2 * WSZ:base + 3 * WSZ] = _sbufpack(
            np.asarray(Wr_down, np.float32)[e])
        gbt[:, 2 + e, :] = _colmaj(br_gate[e])
        ubt[:, 2 + e, :] = _colmaj(br_up[e]) * SXW
    return _bf16(ws), _fp8(we, SWQ), gbt, ubt


def _run(inputs, trace=False):
    x = np.asarray(inputs["x"], np.float32)
    rweights = np.asarray(inputs["routing_weights"], np.float32)
    wsb, we8, gbt, ubt = _pack_shared(
        np.asarray(inputs["Ws_gate"], np.float32), inputs["bs_gate"],
        np.asarray(inputs["Ws_up"], np.float32), inputs["bs_up"],
        np.asarray(inputs["Ws_down"], np.float32), inputs["bs_down"],
        np.asarray(inputs["Wr_gate"], np.float32), inputs["br_gate"],
        np.asarray(inputs["Wr_up"], np.float32), inputs["br_up"],
        np.asarray(inputs["Wr_down"], np.float32), inputs["br_down"])
    bs_down = np.asarray(inputs["bs_down"], np.float32)
    br_down = np.asarray(inputs["br_down"], np.float32)
    # down-bias vector per batch: bs_down + sum_e rw[b,e]*br_down[e]
    cfull = bs_down[None, :] + rweights @ br_down       # [B, D]

    in_maps = []
    for i in range(NCORES):
        xT = x[BL * i:BL * (i + 1)].reshape(T, D).T     # [D, T]
        # pack x to [128, NK, T]: partition p, block k, token t = xT[k*128+p, t]
        xP = xT.reshape(NK, 128, T).transpose(1, 0, 2)
        rwtab = np.empty((128, NU, NTT), np.float32)
        rwtab[:, :2, :] = SDC
        cvtab = np.empty((128, ND, NTT), np.float32)
        for t in range(NTT):
            bg = BL * i + t // (K // TT)
            for e in range(E):
                rwtab[:, 2 + e, t] = rweights[bg, e] * (C / SXW)
            cvtab[:, :, t] = cfull[bg].reshape(ND, 128).T
        in_maps.append({"xTb": _bf16(xP), "xT8": _fp8(xP, SX),
                        "wsb": wsb, "we8": we8,
                        "gb": gbt, "ub": ubt,
                        "rw": np.ascontiguousarray(rwtab),
                        "cv": np.ascontiguousarray(cvtab),
                        "wz": np.zeros((128, 128), ml_dtypes.bfloat16)})

    fused = (not np.any(np.asarray(inputs["bs_up"], np.float32))
             and not np.any(np.asarray(inputs["br_up"], np.float32)))
    res = bass_utils.run_bass_kernel_spmd(_get_nc(fused), in_maps,
                                          core_ids=list(range(NCORES)),
                                          trace=trace)
    out = np.empty((B, K, D), np.float32)
    for i in range(NCORES):
        out[BL * i:BL * (i + 1)] = res.results[i]["outT"].T.reshape(BL, K, D)
    return out, res


def kernel(**inputs) -> np.ndarray:
    out, _ = _run(inputs, trace=False)
    return out

